# revision 1
# baseline (speedup 1.0000x reference)
"""Causal self-attention (B=4, T=2048, D=1024, H=16) on 8 TRN2 NeuronCores.

Sharding: core c handles batch b = c//2 and head-group g = c%2 (8 heads each).
Each core computes, for its (b, g):
    qkv_loc = x[b] @ w_qkv[:, cols(g)]          (q|k|v local, 512 cols each)
    att     = causal_attention(q, k, v)          (8 heads, hd=64)
    y_part  = att @ w_out[rows(g), :]            ([2048, 1024] partial)
Host sums the two partial outputs per batch.

TensorEngine matmuls run in MM dtype (bf16 / f32 / f32r) with fp32 PSUM
accumulation. Softmax uses exp on ScalarE with deferred normalization:
rowsums come free from a ones-column appended to V, and the reciprocal is
broadcast across partitions with a K=1 outer-product matmul.
"""

import os

import numpy as np

import concourse.bass as bass
import concourse.mybir as mybir
from concourse import bacc, tile
from concourse import bass_utils
from concourse.masks import make_identity

# Problem constants (hardcoded per contest contract)
B = 4
T = 2048
D = 1024
H = 16
HD = 64
H_LOC = 8               # heads per core
CLOC = H_LOC * HD       # 512 local head dims
P = 128
N_CORES = 8

F32 = mybir.dt.float32
F32R = mybir.dt.float32r
BF16 = mybir.dt.bfloat16

# Compute dtype knob: "bf16" | "f32" | "f32r"
MM_MODE = os.environ.get("ATTN_MM_MODE", "f32r")
_MM_MAP = {"bf16": BF16, "f32": F32, "f32r": F32R}


def _build_kernel_body(nc, tc, x_ap, wqkv_ap, wout_ap, out_ap, mm):
    from contextlib import ExitStack

    Exp = mybir.ActivationFunctionType.Exp
    mult = mybir.AluOpType.mult

    is_bf16 = mm == BF16
    is_f32r = mm == F32R

    def bitin(ap):
        # DRAM views for direct loads into f32r tiles (bit-identical)
        return ap.bitcast(F32R) if is_f32r else ap

    ctx = ExitStack()
    # ---------------- constants ----------------
    const = ctx.enter_context(tc.tile_pool(name="const", bufs=1))
    ident = const.tile([P, P], mm)
    if is_f32r:
        # gpsimd memset/affine_select can't write f32r; build f32, copy-cast
        scratch = const.tile([P, P], F32, tag="idscratch")
        make_identity(nc, scratch)
        nc.vector.tensor_copy(ident, scratch)
        ones_f = const.tile([P, 16 * H_LOC], F32, tag="ones_f")
        nc.gpsimd.memset(ones_f, 1.0)
    else:
        make_identity(nc, ident)

    # causal mask helper: wm[p, x] = 1.0 iff p <= x - 384 else 0.0
    # (consumed only by DVE multiplies, so f32 is fine in f32r mode)
    wm_dt = BF16 if is_bf16 else F32
    wm = const.tile([P, 896], wm_dt)
    nc.gpsimd.memset(wm, 1.0)
    nc.gpsimd.affine_select(
        out=wm,
        in_=wm,
        compare_op=mybir.AluOpType.is_ge,  # keep where f - p - 384 >= 0
        fill=0.0,
        base=-384,
        channel_multiplier=-1,
        pattern=[[1, 896]],
    )

    oc = const.tile([1, 64], mm)  # ones column for rowsum broadcast
    if is_f32r:
        nc.vector.tensor_copy(oc, ones_f[0:1, 0:64])
    else:
        nc.gpsimd.memset(oc, 1.0)

    qkt_pool = ctx.enter_context(tc.tile_pool(name="qkt", bufs=1))
    QT = qkt_pool.tile([P, 4, T], mm)   # head h -> rows (h%2)*64.., subtile h//2
    KT = qkt_pool.tile([P, 4, T], mm)
    V_aug = qkt_pool.tile([P, 16, H_LOC, HD + 1], mm)  # [j%128, jb, h, dd|ones]
    if is_f32r:
        nc.vector.tensor_copy(
            V_aug[:, :, :, HD],
            ones_f.rearrange("p (a b) -> p a b", a=16),
        )
    else:
        nc.gpsimd.memset(V_aug[:, :, :, HD], 1.0)

    xa = x_ap.rearrange("(tb p) d -> tb p d", p=P)  # [16, 128, 1024]
    wqk = wqkv_ap[:, 0:2 * CLOC].rearrange("(o p) c -> p o c", p=P)
    wv = wqkv_ap[:, 2 * CLOC:3 * CLOC].rearrange("(o p) c -> p o c", p=P)

    ch = CLOC // 2

    def load_cast(pool, shape, tag, src):
        """DMA an f32 DRAM region into an mm-dtype tile."""
        if is_bf16:
            st = pool.tile(shape, F32, tag=tag + "_st")
            nc.sync.dma_start(st, src)
            t = pool.tile(shape, mm, tag=tag)
            nc.vector.tensor_copy(t, st)
            return t
        t = pool.tile(shape, mm, tag=tag)
        nc.sync.dma_start(t, bitin(src))
        return t

    with tc.tile_pool(name="xt", bufs=1) as xt_pool, \
         tc.tile_pool(name="psAB", bufs=2, space="PSUM") as psum:
        xT = xt_pool.tile([P, 8, T], mm)  # [d%128, d//128, t]

        # ---- phase A: x -> xT (transpose+cast), V-proj c-half 0 fused ----
        with tc.tile_pool(name="lda", bufs=2) as lda, \
             tc.tile_pool(name="ldv", bufs=1) as ldv:
            wv_sb = load_cast(ldv, [P, 8, ch], "wv", wv[:, :, 0:ch])
            for tb in range(T // P):
                if is_bf16:
                    xin = lda.tile([P, D], F32, tag="xin")
                    nc.sync.dma_start(xin, xa[tb])
                    xc = lda.tile([P, D], mm, tag="xc")
                    nc.vector.tensor_copy(xc, xin)
                else:
                    xc = lda.tile([P, D], mm, tag="xin")
                    nc.sync.dma_start(xc, bitin(xa[tb]))
                for db in range(D // P):
                    pt = psum.tile([P, P], mm, tag="ps_t")
                    nc.tensor.transpose(pt, xc[:, db * P:(db + 1) * P], ident)
                    nc.vector.tensor_copy(xT[:, db, tb * P:(tb + 1) * P], pt)
                ps = psum.tile([P, ch], F32, tag="ps_v")
                for k in range(8):
                    nc.tensor.matmul(
                        ps,
                        xT[:, k, tb * P:(tb + 1) * P],
                        wv_sb[:, k, :],
                        start=(k == 0),
                        stop=(k == 7),
                    )
                nc.vector.tensor_copy(
                    V_aug[:, tb, 0:4, 0:HD],
                    ps.rearrange("p (h d) -> p h d", h=H_LOC // 2),
                )

        # ---- V-proj c-half 1 ----
        with tc.tile_pool(name="ldv2", bufs=1) as ldv2:
            wv_sb = load_cast(ldv2, [P, 8, ch], "wv2", wv[:, :, ch:2 * ch])
            for tb in range(T // P):
                ps = psum.tile([P, ch], F32, tag="ps_v")
                for k in range(8):
                    nc.tensor.matmul(
                        ps,
                        xT[:, k, tb * P:(tb + 1) * P],
                        wv_sb[:, k, :],
                        start=(k == 0),
                        stop=(k == 7),
                    )
                nc.vector.tensor_copy(
                    V_aug[:, tb, 4:8, 0:HD],
                    ps.rearrange("p (h d) -> p h d", h=H_LOC // 2),
                )

        # ---- phase B: Q^T / K^T proj: psum[c_block 128, t 512] ----
        with tc.tile_pool(name="ldw", bufs=2) as ldw:
            for cb in range(8):
                wcb = load_cast(
                    ldw, [P, 8, P], "wst", wqk[:, :, cb * P:(cb + 1) * P]
                )
                dest = QT if cb < 4 else KT
                sub = cb % 4
                for it in range(4):
                    ps = psum.tile([P, 512], F32, tag="ps_qkv")
                    for k in range(8):
                        nc.tensor.matmul(
                            ps,
                            wcb[:, k, :],
                            xT[:, k, it * 512:(it + 1) * 512],
                            start=(k == 0),
                            stop=(k == 7),
                        )
                    nc.vector.tensor_copy(dest[:, sub, it * 512:(it + 1) * 512], ps)

    # ---------------- phase C: causal attention ----------------
    # Scores matmuls must contract over K=128 partitions: K<96 never warms
    # the PE HAM clock gate (stuck at 1.2 GHz). KT is packed (2 heads = 128
    # real rows) as lhsT; the moving Q operand is a per-parity scratch with
    # the *other* head's 64 rows zeroed, so the packed-KT contraction picks
    # out exactly one head at full K=128.
    atp = ctx.enter_context(tc.tile_pool(name="atp", bufs=1))
    AT = atp.tile([P, 4, T], mm)        # attention output, laid out like QT
    Qp0 = atp.tile([P, T], mm)          # padded Q scratch, even heads
    Qp1 = atp.tile([P, T], mm)          # padded Q scratch, odd heads
    with tc.tile_pool(name="att", bufs=3) as att_pool, \
         tc.tile_pool(name="attsm", bufs=2) as sm_pool, \
         tc.tile_pool(name="psC", bufs=2, space="PSUM") as psum:
        # zero the never-written halves once (x*0 keeps f32r rounding legal)
        nc.vector.tensor_scalar_mul(Qp0[64:128, :], QT[64:128, 0, :], 0.0)
        nc.vector.tensor_scalar_mul(Qp1[0:64, :], QT[0:64, 0, :], 0.0)
        for h in range(H_LOC):
            row0 = (h % 2) * 64
            sub = h // 2
            Qph = Qp0 if h % 2 == 0 else Qp1
            nc.vector.tensor_copy(
                Qph[row0:row0 + 64, :], QT[row0:row0 + 64, sub, :]
            )
            for it in range(4):
                i0 = it * 512
                njb = 4 * (it + 1)
                po = psum.tile([P, 512], F32, tag="ps_o")
                for jb2 in range(njb // 2):
                    ps = psum.tile([P, 1024], F32, tag="ps_s")
                    for u in range(2):
                        jb = 2 * jb2 + u
                        nc.tensor.matmul(
                            ps[:, u * 512:(u + 1) * 512],
                            KT[:, sub, jb * P:(jb + 1) * P],
                            Qph[:, i0:i0 + 512],
                            start=True,
                            stop=True,
                        )
                    es = att_pool.tile([P, 1024], mm, tag="es")
                    nc.scalar.activation(es, ps, Exp, scale=0.125)
                    for u in range(2):
                        jb = 2 * jb2 + u
                        off = jb * P - i0
                        if off >= 0:  # diagonal region: zero out j > i
                            s = 384 - off
                            nc.vector.tensor_tensor(
                                es[:, u * 512:(u + 1) * 512],
                                es[:, u * 512:(u + 1) * 512],
                                wm[:, s:s + 512],
                                mult,
                            )
                    for u in range(2):
                        jb = 2 * jb2 + u
                        nc.tensor.matmul(
                            po[0:HD + 1, :],
                            V_aug[:, jb, h, :],
                            es[:, u * 512:(u + 1) * 512],
                            start=(jb == 0),
                            stop=(jb == njb - 1),
                        )
                # deferred softmax normalization
                rr = sm_pool.tile([1, 512], F32, tag="rr")
                nc.vector.tensor_copy(rr, po[HD:HD + 1, :])
                nc.vector.reciprocal(rr, rr)
                if mm != F32:
                    rm = sm_pool.tile([1, 512], mm, tag="rm")
                    nc.vector.tensor_copy(rm, rr)
                else:
                    rm = rr
                pb = psum.tile([64, 512], F32, tag="ps_b")
                nc.tensor.matmul(pb, oc, rm, start=True, stop=True)
                rb = sm_pool.tile([64, 512], F32, tag="rb")
                nc.vector.tensor_copy(rb, pb)
                nc.vector.tensor_tensor(
                    AT[row0:row0 + 64, sub, i0:i0 + 512],
                    po[0:64, :],
                    rb,
                    mult,
                )

    # ---------------- phase D: output projection ----------------
    wo = wout_ap.rearrange("(o p) n -> p o n", p=P)  # [128, 4, 1024]
    oa = out_ap.rearrange("(tb p) d -> tb p d", p=P)
    with tc.tile_pool(name="ldo", bufs=2) as ldo, \
         tc.tile_pool(name="ypool", bufs=3) as ypool, \
         tc.tile_pool(name="psD", bufs=4, space="PSUM") as psum:
        if is_bf16:
            wo_st = ldo.tile([P, 4, D], F32, tag="wo_st")
            nc.sync.dma_start(wo_st, wo)
            wo_sb = ldo.tile([P, 4, D], mm, tag="wo_sb")
            nc.vector.tensor_copy(wo_sb, wo_st)
        else:
            wo_sb = ldo.tile([P, 4, D], mm, tag="wo_st")
            nc.sync.dma_start(wo_sb, bitin(wo))
        for tb in range(T // P):
            for nt in range(2):
                py = psum.tile([P, 512], F32, tag="ps_y")
                for k in range(4):
                    nc.tensor.matmul(
                        py,
                        AT[:, k, tb * P:(tb + 1) * P],
                        wo_sb[:, k, nt * 512:(nt + 1) * 512],
                        start=(k == 0),
                        stop=(k == 3),
                    )
                ysb = ypool.tile([P, 512], F32, tag="ysb")
                nc.vector.tensor_copy(ysb, py)
                nc.sync.dma_start(oa[tb, :, nt * 512:(nt + 1) * 512], ysb)

    ctx.close()


_CACHE = {}


def _get_nc(mode=None):
    mode = mode or MM_MODE
    if mode in _CACHE:
        return _CACHE[mode]
    mm = _MM_MAP[mode]
    nc = bacc.Bacc(
        "TRN2",
        target_bir_lowering=False,
        debug=False,
        enable_asserts=False,
        num_devices=N_CORES,
    )
    x_d = nc.dram_tensor("x", [T, D], F32, kind="ExternalInput")
    wqkv_d = nc.dram_tensor("w_qkv", [D, 3 * CLOC], F32, kind="ExternalInput")
    wout_d = nc.dram_tensor("w_out", [CLOC, D], F32, kind="ExternalInput")
    out_d = nc.dram_tensor("out", [T, D], F32, kind="ExternalOutput")
    with tile.TileContext(nc) as tc:
        _build_kernel_body(
            nc, tc, x_d.ap(), wqkv_d.ap(), wout_d.ap(), out_d.ap(), mm
        )
    nc.compile()
    _CACHE[mode] = nc
    return nc


def _make_in_maps(x, w_qkv, w_out):
    x = np.ascontiguousarray(np.asarray(x, dtype=np.float32))
    w_qkv = np.ascontiguousarray(np.asarray(w_qkv, dtype=np.float32))
    w_out = np.ascontiguousarray(np.asarray(w_out, dtype=np.float32))
    in_maps = []
    for c in range(N_CORES):
        b, g = divmod(c, 2)
        c0 = g * CLOC
        wloc = np.concatenate(
            [
                w_qkv[:, c0:c0 + CLOC],
                w_qkv[:, D + c0:D + c0 + CLOC],
                w_qkv[:, 2 * D + c0:2 * D + c0 + CLOC],
            ],
            axis=1,
        )
        in_maps.append({
            "x": np.ascontiguousarray(x[b]),
            "w_qkv": np.ascontiguousarray(wloc),
            "w_out": np.ascontiguousarray(w_out[c0:c0 + CLOC]),
        })
    return in_maps


def run(x, w_qkv, w_out, trace=False, mode=None):
    nc = _get_nc(mode)
    in_maps = _make_in_maps(x, w_qkv, w_out)
    res = bass_utils.run_bass_kernel_spmd(
        nc, in_maps, core_ids=list(range(N_CORES)), trace=trace
    )
    y = np.empty((B, T, D), dtype=np.float32)
    for b in range(B):
        y[b] = res.results[2 * b]["out"] + res.results[2 * b + 1]["out"]
    return y, res


def kernel(x, w_qkv, w_out):
    y, _ = run(x, w_qkv, w_out, trace=False)
    return y



# revision 21
# speedup vs baseline: 1.8140x; 1.8140x over previous
"""Causal self-attention (B=4, T=2048, D=1024, H=16) on 8 TRN2 NeuronCores.

Sharding: core c handles batch b = c//2 and head-group g = c%2 (8 heads each).
Each core computes, for its (b, g):
    qkv_loc = x[b] @ w_qkv[:, cols(g)]          (q|k|v local, 512 cols each)
    att     = causal_attention(q, k, v)          (8 heads, hd=64)
    y_part  = att @ w_out[rows(g), :]            ([2048, 1024] partial)
Host sums the two partial outputs per batch.

All matmuls run in bf16 (host-cast inputs) with fp32 PSUM accumulation.
The attention phase is software-pipelined: AV matmuls lag the score
matmuls by AV_LAG pairs so the in-order PE queue never waits on
exp/mask; QK-projection work for the *next* head-pair is woven between
attention pairs so PE stalls caused by the scalar engine (exp) are
filled with projection matmuls. Softmax normalization is deferred
(rowsums ride along as a ones-column of V); the reciprocal uses the
fast DVE approximation and the partition broadcast runs on the
otherwise-idle GpSimd engine.
"""

from collections import deque

import numpy as np
import ml_dtypes

import concourse.bass as bass
import concourse.mybir as mybir
from concourse import bacc, tile
from concourse import bass_utils
from concourse.masks import make_identity

# Problem constants (hardcoded per contest contract)
B = 4
T = 2048
D = 1024
H = 16
HD = 64
H_LOC = 8               # heads per core
CLOC = H_LOC * HD       # 512 local head dims
P = 128
N_CORES = 8
NTB = T // P            # 16 t-blocks

F32 = mybir.dt.float32
BF16 = mybir.dt.bfloat16

AV_LAG = 4              # pairs the AV matmuls lag behind the score matmuls
NORM_A_LAG = 2          # av-pops before emitting recip+broadcast
NORM_B_LAG = 5          # av-pops before emitting the normalize multiply
WEAVE_EVERY = 5         # emit one projection it-unit per N attention pairs

Exp = mybir.ActivationFunctionType.Exp
MULT = mybir.AluOpType.mult


def _build_kernel_body(nc, tc, x_ap, wqkv_ap, wout_ap, out_ap, dbg=None):
    from contextlib import ExitStack

    ctx = ExitStack()

    # ---------------- constants ----------------
    const = ctx.enter_context(tc.tile_pool(name="const", bufs=1))
    ident = const.tile([P, P], BF16)
    make_identity(nc, ident)

    # causal mask helper: wm[p, x] = 1.0 iff p <= x - 384 else 0.0
    wm = const.tile([P, 896], BF16)
    nc.gpsimd.memset(wm, 1.0)
    nc.gpsimd.affine_select(
        out=wm,
        in_=wm,
        compare_op=mybir.AluOpType.is_ge,  # keep where f - p - 384 >= 0
        fill=0.0,
        base=-384,
        channel_multiplier=-1,
        pattern=[[1, 896]],
    )

    big = ctx.enter_context(tc.tile_pool(name="big", bufs=1))
    xT = big.tile([P, 8, T], BF16)                 # [d%128, d//128, t]
    V_aug = big.tile([P, NTB, H_LOC, HD + 1], BF16)  # [j%128, jb, h, dd|1]
    nc.gpsimd.memset(V_aug[:, :, :, HD], 1.0)
    AT = big.tile([P, 4, T], BF16)                 # [(h%2)*64+dd, h//2, t]
    wo_sb = big.tile([P, 4, D], BF16, tag="wo")    # out-proj weights
    # double-buffered per-segment score operands
    kt = [big.tile([P, T], BF16, tag=f"kt{i}", name=f"kt{i}") for i in range(2)]
    qpe = [big.tile([P, T], BF16, tag=f"qpe{i}", name=f"qpe{i}") for i in range(2)]
    qpo = [big.tile([P, T], BF16, tag=f"qpo{i}", name=f"qpo{i}") for i in range(2)]
    for i in range(2):
        nc.gpsimd.memset(qpe[i][64:128, :], 0.0)
        nc.gpsimd.memset(qpo[i][0:64, :], 0.0)

    xa = x_ap.rearrange("(tb p) d -> tb p d", p=P)        # [16, 128, 1024]
    wqk = wqkv_ap[:, 0:2 * CLOC].rearrange("(o p) c -> p o c", p=P)
    wv = wqkv_ap[:, 2 * CLOC:3 * CLOC].rearrange("(o p) c -> p o c", p=P)

    # ---- phase A: x -> xT (transpose), V projection ----
    with tc.tile_pool(name="lda", bufs=3) as lda, \
         tc.tile_pool(name="ldv", bufs=1) as ldv, \
         tc.tile_pool(name="psA", bufs=2, space="PSUM") as psA:
        wv_sb = ldv.tile([P, 8, CLOC], BF16, tag="wv")

        def emit_vproj(tb):
            ps = psA.tile([P, CLOC], F32, tag="psv")
            for k in range(8):
                nc.tensor.matmul(
                    ps,
                    xT[:, k, tb * P:(tb + 1) * P],
                    wv_sb[:, k, :],
                    start=(k == 0),
                    stop=(k == 7),
                )
            nc.vector.tensor_copy(
                V_aug[:, tb, :, 0:HD],
                ps.rearrange("p (h d) -> p h d", h=H_LOC),
            )

        for tb in range(NTB):
            xc = lda.tile([P, D], BF16, tag="xin")
            nc.sync.dma_start(xc, xa[tb])
            if tb == 0:
                nc.sync.dma_start(wv_sb, wv)
            for g in range(2):
                pt = psA.tile([P, 512], BF16, tag="pt")
                for j in range(4):
                    nc.tensor.transpose(
                        pt[:, j * P:(j + 1) * P],
                        xc[:, (g * 4 + j) * P:(g * 4 + j + 1) * P],
                        ident,
                    )
                nc.vector.tensor_copy(
                    xT[:, g * 4:(g + 1) * 4, tb * P:(tb + 1) * P],
                    pt.rearrange("p (a b) -> p a b", a=4),
                )
            if tb > 0:
                emit_vproj(tb - 1)
        emit_vproj(NTB - 1)
        # prefetch out-proj weights while the DMA engine is otherwise idle
        nc.sync.dma_start(wo_sb, wout_ap.rearrange("(o p) n -> p o n", p=P))

    # ---- interleaved phases B (QK projection) + C (attention) ----
    # Work queues for software pipelining.
    av_q = deque()        # pending AV-pair closures
    norm_q = []           # [countdown, closure] normalize steps
    b_q = deque()         # pending projection it-unit closures

    def pop_norms():
        for e in norm_q:
            e[0] -= 1
        while norm_q and norm_q[0][0] <= 0:
            norm_q.pop(0)[1]()

    def pop_av(n=1):
        for _ in range(n):
            if av_q:
                av_q.popleft()()
                pop_norms()

    def pop_b(n=1):
        for _ in range(n):
            if b_q:
                b_q.popleft()()

    with tc.tile_pool(name="ldw", bufs=4) as ldw, \
         tc.tile_pool(name="att", bufs=1) as att_pool, \
         tc.tile_pool(name="sm", bufs=2) as sm_pool, \
         tc.tile_pool(name="psBC", bufs=1, space="PSUM") as psum:

        norm_idx = [0]

        # --- phase B segment: project q (cb=p) and k (cb=4+p) for pair p ---
        def emit_b_segment(p):
            """Queue 8 it-units computing Qp/KT for head-pair p."""
            buf = p % 2
            for qk in range(2):            # 0 = q columns, 1 = k columns
                cb = p + 4 * qk
                wcb = ldw.tile([P, 8, P], BF16, tag="wst")
                nc.sync.dma_start(wcb, wqk[:, :, cb * P:(cb + 1) * P])

                def unit(it, qk=qk, wcb=wcb, buf=buf):
                    ps = psum.tile([P, 512], F32, tag="ps_qkv")
                    for k in range(8):
                        nc.tensor.matmul(
                            ps,
                            wcb[:, k, :],
                            xT[:, k, it * 512:(it + 1) * 512],
                            start=(k == 0),
                            stop=(k == 7),
                        )
                    sl = slice(it * 512, (it + 1) * 512)
                    if qk == 0:
                        nc.vector.tensor_copy(qpe[buf][0:64, sl], ps[0:64, :])
                        nc.vector.tensor_copy(qpo[buf][64:128, sl], ps[64:128, :])
                    else:
                        nc.vector.tensor_copy(kt[buf][:, sl], ps)

                for it in range(4):
                    b_q.append(lambda it=it, unit=unit: unit(it))

        # --- phase C segment: attention for heads 2p, 2p+1 ---
        def emit_c_segment(p):
            buf = p % 2
            ktb = kt[buf]
            for u2 in range(2):
                h = 2 * p + u2
                hp = qpe[buf] if u2 == 0 else qpo[buf]
                row0 = u2 * 64
                for it in range(4):
                    i0 = it * 512
                    njb = 4 * (it + 1)
                    po = psum.tile([P, 512], F32, tag="ps_o", bufs=3)
                    for jb2 in range(njb // 2):
                        ps = psum.tile([P, 1024], F32, tag="ps_s", bufs=2)
                        for u in range(2):
                            jb = 2 * jb2 + u
                            nc.tensor.matmul(
                                ps[:, u * 512:(u + 1) * 512],
                                ktb[:, jb * P:(jb + 1) * P],
                                hp[:, i0:i0 + 512],
                                start=True,
                                stop=True,
                            )
                        es = att_pool.tile([P, 1024], BF16, tag="es", bufs=6)
                        nc.scalar.activation(es, ps, Exp, scale=0.125)
                        for u in range(2):
                            jb = 2 * jb2 + u
                            off = jb * P - i0
                            if off >= 0:  # diagonal region: zero out j > i
                                s = 384 - off
                                nc.vector.tensor_tensor(
                                    es[:, u * 512:(u + 1) * 512],
                                    es[:, u * 512:(u + 1) * 512],
                                    wm[:, s:s + 512],
                                    MULT,
                                )

                        def av(jb2=jb2, es=es, po=po, h=h, njb=njb,
                               row0=row0, p=p, i0=i0, it=it, last=(jb2 == njb // 2 - 1)):
                            for u in range(2):
                                jb = 2 * jb2 + u
                                nc.tensor.matmul(
                                    po[0:HD + 1, :],
                                    V_aug[:, jb, h, :],
                                    es[:, u * 512:(u + 1) * 512],
                                    start=(jb == 0),
                                    stop=(jb == njb - 1),
                                )
                            if last:
                                def norm_a(po=po):
                                    n = norm_idx[0]
                                    norm_idx[0] += 1
                                    rs = sm_pool.tile([1, 512], F32, tag="rs")
                                    rinv = sm_pool.tile([1, 512], F32, tag="rinv")
                                    nc.vector.tensor_copy(rs, po[HD:HD + 1, :])
                                    nc.vector.reciprocal_approx_fast(rinv, rs)
                                    rbb = sm_pool.tile([64, 512], F32, tag="rbb")
                                    nc.gpsimd.partition_broadcast(rbb, rinv)
                                    norm_a.rbb = rbb
                                    if dbg is not None:
                                        nc.sync.dma_start(
                                            dbg["rs_all"].ap()[n], rs
                                        )
                                        nc.sync.dma_start(
                                            dbg["rinv_all"].ap()[n], rinv
                                        )

                                def norm_b(po=po, row0=row0, p=p, i0=i0):
                                    nc.vector.tensor_tensor(
                                        AT[row0:row0 + 64, p, i0:i0 + 512],
                                        po[0:HD, :],
                                        norm_a.rbb,
                                        MULT,
                                    )

                                norm_q.append([NORM_A_LAG, norm_a])
                                norm_q.append([NORM_B_LAG, norm_b])

                        av_q.append(av)
                        while len(av_q) > AV_LAG:
                            pop_av()
                        # weave projection work for the next segment
                        emit_c_segment.ctr += 1
                        if emit_c_segment.ctr % WEAVE_EVERY == 0:
                            pop_b()

        emit_c_segment.ctr = 0

        emit_b_segment(0)
        pop_b(8)                      # first segment's projections up front
        for p in range(4):
            if p < 3:
                emit_b_segment(p + 1)
            emit_c_segment(p)
            pop_b(8)                  # drain any unwoven projection units
        while av_q:
            pop_av()
        while norm_q:
            norm_q.pop(0)[1]()

    # ---------------- phase D: output projection ----------------
    oa = out_ap.rearrange("(tb p) d -> tb p d", p=P)
    with tc.tile_pool(name="ypool", bufs=3) as ypool, \
         tc.tile_pool(name="psD", bufs=4, space="PSUM") as psD:
        for tb in range(NTB):
            for nt in range(2):
                py = psD.tile([P, 512], F32, tag="ps_y")
                for k in range(4):
                    nc.tensor.matmul(
                        py,
                        AT[:, k, tb * P:(tb + 1) * P],
                        wo_sb[:, k, nt * 512:(nt + 1) * 512],
                        start=(k == 0),
                        stop=(k == 3),
                    )
                ysb = ypool.tile([P, 512], F32, tag="ysb")
                nc.vector.tensor_copy(ysb, py)
                nc.sync.dma_start(oa[tb, :, nt * 512:(nt + 1) * 512], ysb)

    if dbg is not None:
        nc.sync.dma_start(dbg["xT"].ap(), xT)
        nc.sync.dma_start(dbg["V_aug"].ap(), V_aug)
        nc.sync.dma_start(dbg["AT"].ap(), AT)
        for i in range(2):
            nc.sync.dma_start(dbg[f"kt{i}"].ap(), kt[i])
            nc.sync.dma_start(dbg[f"qpe{i}"].ap(), qpe[i])
            nc.sync.dma_start(dbg[f"qpo{i}"].ap(), qpo[i])

    ctx.close()


_CACHE = {}


def _get_nc(debug=False):
    key = ("nc", debug)
    if key in _CACHE:
        return _CACHE[key]
    nc = bacc.Bacc(
        "TRN2",
        target_bir_lowering=False,
        debug=False,
        enable_asserts=False,
        num_devices=N_CORES,
    )
    x_d = nc.dram_tensor("x", [T, D], BF16, kind="ExternalInput")
    wqkv_d = nc.dram_tensor("w_qkv", [D, 3 * CLOC], BF16, kind="ExternalInput")
    wout_d = nc.dram_tensor("w_out", [CLOC, D], BF16, kind="ExternalInput")
    out_d = nc.dram_tensor("out", [T, D], F32, kind="ExternalOutput")
    dbg = None
    if debug:
        dbg = {
            "xT": nc.dram_tensor("dbg_xT", [P, 8, T], BF16, kind="ExternalOutput"),
            "V_aug": nc.dram_tensor("dbg_V_aug", [P, NTB, H_LOC, HD + 1], BF16, kind="ExternalOutput"),
            "AT": nc.dram_tensor("dbg_AT", [P, 4, T], BF16, kind="ExternalOutput"),
        }
        for i in range(2):
            dbg[f"kt{i}"] = nc.dram_tensor(f"dbg_kt{i}", [P, T], BF16, kind="ExternalOutput")
            dbg[f"qpe{i}"] = nc.dram_tensor(f"dbg_qpe{i}", [P, T], BF16, kind="ExternalOutput")
            dbg[f"qpo{i}"] = nc.dram_tensor(f"dbg_qpo{i}", [P, T], BF16, kind="ExternalOutput")
        dbg["rs_all"] = nc.dram_tensor("dbg_rs_all", [32, 1, 512], F32, kind="ExternalOutput")
        dbg["rinv_all"] = nc.dram_tensor("dbg_rinv_all", [32, 1, 512], F32, kind="ExternalOutput")
    with tile.TileContext(nc) as tc:
        _build_kernel_body(
            nc, tc, x_d.ap(), wqkv_d.ap(), wout_d.ap(), out_d.ap(), dbg
        )
    nc.compile()
    _CACHE[key] = nc
    return nc


def _make_in_maps(x, w_qkv, w_out):
    bf = ml_dtypes.bfloat16
    x = np.asarray(x, dtype=np.float32)
    w_qkv = np.asarray(w_qkv, dtype=np.float32)
    w_out = np.asarray(w_out, dtype=np.float32)
    in_maps = []
    for c in range(N_CORES):
        b, g = divmod(c, 2)
        c0 = g * CLOC
        wloc = np.concatenate(
            [
                w_qkv[:, c0:c0 + CLOC],
                w_qkv[:, D + c0:D + c0 + CLOC],
                w_qkv[:, 2 * D + c0:2 * D + c0 + CLOC],
            ],
            axis=1,
        )
        in_maps.append({
            "x": np.ascontiguousarray(x[b]).astype(bf),
            "w_qkv": np.ascontiguousarray(wloc).astype(bf),
            "w_out": np.ascontiguousarray(w_out[c0:c0 + CLOC]).astype(bf),
        })
    return in_maps


def run(x, w_qkv, w_out, trace=False, debug=False):
    nc = _get_nc(debug)
    in_maps = _make_in_maps(x, w_qkv, w_out)
    res = bass_utils.run_bass_kernel_spmd(
        nc, in_maps, core_ids=list(range(N_CORES)), trace=trace
    )
    y = np.empty((B, T, D), dtype=np.float32)
    for b in range(B):
        y[b] = res.results[2 * b]["out"] + res.results[2 * b + 1]["out"]
    return y, res


def kernel(x, w_qkv, w_out):
    y, _ = run(x, w_qkv, w_out, trace=False)
    return y


# revision 35
# speedup vs baseline: 2.0690x; 1.1406x over previous
"""Causal self-attention (B=4, T=2048, D=1024, H=16) on 8 TRN2 NeuronCores.

Sharding: core c handles batch b = c//2 and head-group g = c%2 (8 heads each).
Each core computes, for its (b, g):
    qkv_loc = x[b] @ w_qkv[:, cols(g)]          (q|k|v local, 512 cols each)
    att     = causal_attention(q, k, v)          (8 heads, hd=64)
    y_part  = att @ w_out[rows(g), :]            ([2048, 1024] partial)
Host sums the two partial outputs per batch.

All matmuls run in bf16 (host-cast inputs) with fp32 PSUM accumulation.
The attention phase is software-pipelined: AV matmuls lag the score
matmuls by AV_LAG pairs so the in-order PE queue never waits on
exp/mask; QK-projection work for the *next* head-pair is woven between
attention pairs so PE stalls caused by the scalar engine (exp) are
filled with projection matmuls. Softmax normalization is deferred
(rowsums ride along as a ones-column of V); the reciprocal uses the
fast DVE approximation and the partition broadcast runs on the
otherwise-idle GpSimd engine.
"""

from collections import deque

import numpy as np
import ml_dtypes

import concourse.bass as bass
import concourse.mybir as mybir
from concourse import bacc, tile
from concourse import bass_utils
from concourse.masks import make_identity

# Problem constants (hardcoded per contest contract)
B = 4
T = 2048
D = 1024
H = 16
HD = 64
H_LOC = 8               # heads per core
CLOC = H_LOC * HD       # 512 local head dims
P = 128
N_CORES = 8
NTB = T // P            # 16 t-blocks

F32 = mybir.dt.float32
BF16 = mybir.dt.bfloat16

AV_LAG = 4              # pairs the AV matmuls lag behind the score matmuls
NORM_A_LAG = 2          # av-pops before emitting recip+broadcast
NORM_B_LAG = 5          # av-pops before emitting the normalize multiply
WEAVE_EVERY = 5         # emit one projection it-unit per N attention pairs

Exp = mybir.ActivationFunctionType.Exp
MULT = mybir.AluOpType.mult


def _build_kernel_body(nc, tc, x_ap, wqkv_ap, wout_ap, out_ap, dbg=None):
    from contextlib import ExitStack

    ctx = ExitStack()

    # ---------------- constants ----------------
    const = ctx.enter_context(tc.tile_pool(name="const", bufs=1))
    # causal mask helper: wm[p, x] = 1.0 iff p <= x - 384 else 0.0
    wm = const.tile([P, 896], BF16)
    nc.gpsimd.memset(wm, 1.0)
    nc.gpsimd.affine_select(
        out=wm,
        in_=wm,
        compare_op=mybir.AluOpType.is_ge,  # keep where f - p - 384 >= 0
        fill=0.0,
        base=-384,
        channel_multiplier=-1,
        pattern=[[1, 896]],
    )

    big = ctx.enter_context(tc.tile_pool(name="big", bufs=1))
    xT = big.tile([P, 8, T], BF16)                 # [d%128, d//128, t]
    V_aug = big.tile([P, NTB, H_LOC, HD + 1], BF16)  # [j%128, jb, h, dd|1]
    nc.gpsimd.memset(V_aug[:, :, :, HD], 1.0)
    AT = big.tile([P, 4, T], BF16)                 # [(h%2)*64+dd, h//2, t]
    wo_sb = big.tile([P, 4, D], BF16, tag="wo")    # out-proj weights
    # double-buffered per-segment score operands
    kt = [big.tile([P, T], BF16, tag=f"kt{i}", name=f"kt{i}") for i in range(2)]
    qpe = [big.tile([P, T], BF16, tag=f"qpe{i}", name=f"qpe{i}") for i in range(2)]
    qpo = [big.tile([P, T], BF16, tag=f"qpo{i}", name=f"qpo{i}") for i in range(2)]
    for i in range(2):
        nc.gpsimd.memset(qpe[i][64:128, :], 0.0)
        nc.gpsimd.memset(qpo[i][0:64, :], 0.0)

    wv_sb = big.tile([P, 8, CLOC], BF16, tag="wv")

    wqk = wqkv_ap[:, 0:2 * CLOC].rearrange("(o p) c -> p o c", p=P)
    wv = wqkv_ap[:, 2 * CLOC:3 * CLOC].rearrange("(o p) c -> p o c", p=P)

    # ---- interleaved phases A (x load + V proj) + B (QK proj) + C ----
    # Work queues for software pipelining.
    av_q = deque()        # pending AV-pair closures
    norm_q = []           # [countdown, closure] normalize steps
    b_q = deque()         # pending projection it-unit closures

    def pop_norms():
        for e in norm_q:
            e[0] -= 1
        while norm_q and norm_q[0][0] <= 0:
            norm_q.pop(0)[1]()

    def pop_av(n=1):
        for _ in range(n):
            if av_q:
                av_q.popleft()()
                pop_norms()

    emitted = set()

    def pop_b(n=1):
        for _ in range(n):
            if b_q:
                key, fn = b_q.popleft()
                fn()
                emitted.add(key)

    def need(keys):
        while not all(k in emitted for k in keys):
            assert b_q, f"missing producer units: {[k for k in keys if k not in emitted]}"
            pop_b()

    xt_dram = x_ap                     # host pre-transposed: [128, 8, 2048]

    with tc.tile_pool(name="ldw", bufs=4) as ldw, \
         tc.tile_pool(name="att", bufs=1) as att_pool, \
         tc.tile_pool(name="sm", bufs=2) as sm_pool, \
         tc.tile_pool(name="psBC", bufs=1, space="PSUM") as psum:

        norm_idx = [0]

        def v_unit(tb):
            ps = psum.tile([P, CLOC], F32, tag="ps_qkv", bufs=2)
            for k in range(8):
                nc.tensor.matmul(
                    ps,
                    xT[:, k, tb * P:(tb + 1) * P],
                    wv_sb[:, k, :],
                    start=(k == 0),
                    stop=(k == 7),
                )
            nc.vector.tensor_copy(
                V_aug[:, tb, :, 0:HD],
                ps.rearrange("p (h d) -> p h d", h=H_LOC),
            )

        # --- phase B segment: project q (cb=p) and k (cb=4+p) for pair p ---
        def emit_b_segment(p, with_v=False):
            """Queue it-units computing Qp/KT (and V for segment 0)."""
            buf = p % 2
            wcbs = []
            for qk in range(2):            # 0 = q columns, 1 = k columns
                cb = p + 4 * qk
                wcb = ldw.tile([P, 8, P], BF16, tag="wst", name="wcb")
                nc.sync.dma_start(wcb, wqk[:, :, cb * P:(cb + 1) * P])
                wcbs.append(wcb)

            def unit(it, qk):
                wcb = wcbs[qk]
                ps = psum.tile([P, 512], F32, tag="ps_qkv", bufs=2)
                for k in range(8):
                    nc.tensor.matmul(
                        ps,
                        wcb[:, k, :],
                        xT[:, k, it * 512:(it + 1) * 512],
                        start=(k == 0),
                        stop=(k == 7),
                    )
                sl = slice(it * 512, (it + 1) * 512)
                if qk == 0:
                    nc.vector.tensor_copy(qpe[buf][0:64, sl], ps[0:64, :])
                    nc.vector.tensor_copy(qpo[buf][64:128, sl], ps[64:128, :])
                else:
                    nc.vector.tensor_copy(kt[buf][:, sl], ps)

            for it in range(4):
                for qk in range(2):
                    b_q.append((("b", p, qk, it),
                                lambda it=it, qk=qk: unit(it, qk)))
                if with_v:
                    for tb in range(4 * it, 4 * it + 4):
                        b_q.append((("v", tb), lambda tb=tb: v_unit(tb)))

        # --- phase C segment: attention for heads 2p, 2p+1 ---
        def emit_c_segment(p):
            buf = p % 2
            ktb = kt[buf]
            for u2 in range(2):
                h = 2 * p + u2
                hp = qpe[buf] if u2 == 0 else qpo[buf]
                row0 = u2 * 64
                for it in range(4):
                    i0 = it * 512
                    njb = 4 * (it + 1)
                    req = [("b", p, 0, it)] + [("b", p, 1, j) for j in range(it + 1)]
                    if p == 0:
                        req += [("v", tb) for tb in range(njb)]
                    need(req)
                    po = psum.tile([P, 512], F32, tag="ps_o", bufs=2)
                    for jb2 in range(njb // 2):
                        # causal narrowing: diagonal block jb covers only
                        # columns [off, 512) of the i-window (off = jb*128-i0)
                        offs = [max(0, (2 * jb2 + u) * P - i0) for u in range(2)]
                        ps = psum.tile([P, 1024], F32, tag="ps_s", bufs=2)
                        for u in range(2):
                            jb = 2 * jb2 + u
                            off = offs[u]
                            nc.tensor.matmul(
                                ps[:, u * 512 + off:(u + 1) * 512],
                                ktb[:, jb * P:(jb + 1) * P],
                                hp[:, i0 + off:i0 + 512],
                                start=True,
                                stop=True,
                            )
                        es = att_pool.tile([P, 1024], BF16, tag="es", bufs=8)
                        if offs[0] >= 256:
                            # widest-masked pair: exp each block separately
                            for u in range(2):
                                off = offs[u]
                                nc.scalar.activation(
                                    es[:, u * 512 + off:(u + 1) * 512],
                                    ps[:, u * 512 + off:(u + 1) * 512],
                                    Exp, scale=0.125,
                                )
                        else:
                            nc.scalar.activation(es, ps, Exp, scale=0.125)
                        for u in range(2):
                            off = offs[u]
                            if 2 * jb2 + u >= njb - 4:  # diagonal region
                                nc.vector.tensor_tensor(
                                    es[:, u * 512 + off:(u + 1) * 512],
                                    es[:, u * 512 + off:(u + 1) * 512],
                                    wm[:, 384:896 - off],
                                    MULT,
                                )

                        def av(jb2=jb2, es=es, po=po, h=h, njb=njb, offs=offs,
                               row0=row0, p=p, i0=i0, it=it, last=(jb2 == njb // 2 - 1)):
                            for u in range(2):
                                jb = 2 * jb2 + u
                                off = offs[u]
                                nc.tensor.matmul(
                                    po[0:HD + 1, off:512],
                                    V_aug[:, jb, h, :],
                                    es[:, u * 512 + off:(u + 1) * 512],
                                    start=(jb == 0),
                                    stop=(jb == njb - 1),
                                )
                            if last:
                                def norm_a(po=po):
                                    n = norm_idx[0]
                                    norm_idx[0] += 1
                                    rs = sm_pool.tile([1, 512], F32, tag="rs")
                                    rinv = sm_pool.tile([1, 512], F32, tag="rinv")
                                    nc.vector.tensor_copy(rs, po[HD:HD + 1, :])
                                    nc.vector.reciprocal_approx_fast(rinv, rs)
                                    rbb = sm_pool.tile([64, 512], F32, tag="rbb")
                                    nc.gpsimd.partition_broadcast(rbb, rinv)
                                    norm_a.rbb = rbb
                                    if dbg is not None:
                                        nc.sync.dma_start(
                                            dbg["rs_all"].ap()[n], rs
                                        )
                                        nc.sync.dma_start(
                                            dbg["rinv_all"].ap()[n], rinv
                                        )

                                def norm_b(po=po, row0=row0, p=p, i0=i0):
                                    nc.vector.tensor_tensor(
                                        AT[row0:row0 + 64, p, i0:i0 + 512],
                                        po[0:HD, :],
                                        norm_a.rbb,
                                        MULT,
                                    )

                                norm_q.append([NORM_A_LAG, norm_a])
                                norm_q.append([NORM_B_LAG, norm_b])

                        av_q.append(av)
                        while len(av_q) > AV_LAG:
                            pop_av()
                        # weave projection work between attention pairs
                        emit_c_segment.ctr += 1
                        if len(b_q) > 10 or (emit_c_segment.ctr % WEAVE_EVERY == 0):
                            pop_b()

        emit_c_segment.ctr = 0

        # stream in x^T (host pre-transposed) by i-chunk; queue segment-0
        # projection + V units, then run the pipelined B/C schedule
        emit_b_segment(0, with_v=True)
        for itc in range(4):
            nc.sync.dma_start(
                xT[:, :, itc * 512:(itc + 1) * 512],
                xt_dram[:, :, itc * 512:(itc + 1) * 512],
            )
            if itc == 0:
                nc.sync.dma_start(wv_sb, wv)
        nc.sync.dma_start(wo_sb, wout_ap.rearrange("(o p) n -> p o n", p=P))
        for p in range(4):
            if p < 3:
                emit_b_segment(p + 1)
            emit_c_segment(p)
        pop_b(len(b_q))
        while av_q:
            pop_av()
        while norm_q:
            norm_q.pop(0)[1]()

    # ---------------- phase D: output projection ----------------
    oa = out_ap.rearrange("(tb p) d -> tb p d", p=P)
    with tc.tile_pool(name="ypool", bufs=3) as ypool, \
         tc.tile_pool(name="psD", bufs=4, space="PSUM") as psD:
        for tb in range(NTB):
            for nt in range(2):
                py = psD.tile([P, 512], F32, tag="ps_y")
                for k in range(4):
                    nc.tensor.matmul(
                        py,
                        AT[:, k, tb * P:(tb + 1) * P],
                        wo_sb[:, k, nt * 512:(nt + 1) * 512],
                        start=(k == 0),
                        stop=(k == 3),
                    )
                ysb = ypool.tile([P, 512], BF16, tag="ysb")
                nc.vector.tensor_copy(ysb, py)
                nc.sync.dma_start(oa[tb, :, nt * 512:(nt + 1) * 512], ysb)

    if dbg is not None:
        nc.sync.dma_start(dbg["xT"].ap(), xT)
        nc.sync.dma_start(dbg["V_aug"].ap(), V_aug)
        nc.sync.dma_start(dbg["AT"].ap(), AT)
        for i in range(2):
            nc.sync.dma_start(dbg[f"kt{i}"].ap(), kt[i])
            nc.sync.dma_start(dbg[f"qpe{i}"].ap(), qpe[i])
            nc.sync.dma_start(dbg[f"qpo{i}"].ap(), qpo[i])

    ctx.close()


_CACHE = {}


def _get_nc(debug=False):
    key = ("nc", debug)
    if key in _CACHE:
        return _CACHE[key]
    nc = bacc.Bacc(
        "TRN2",
        target_bir_lowering=False,
        debug=False,
        enable_asserts=False,
        num_devices=N_CORES,
    )
    x_d = nc.dram_tensor("x", [P, 8, T], BF16, kind="ExternalInput")
    wqkv_d = nc.dram_tensor("w_qkv", [D, 3 * CLOC], BF16, kind="ExternalInput")
    wout_d = nc.dram_tensor("w_out", [CLOC, D], BF16, kind="ExternalInput")
    out_d = nc.dram_tensor("out", [T, D], BF16, kind="ExternalOutput")
    dbg = None
    if debug:
        dbg = {
            "xT": nc.dram_tensor("dbg_xT", [P, 8, T], BF16, kind="ExternalOutput"),
            "V_aug": nc.dram_tensor("dbg_V_aug", [P, NTB, H_LOC, HD + 1], BF16, kind="ExternalOutput"),
            "AT": nc.dram_tensor("dbg_AT", [P, 4, T], BF16, kind="ExternalOutput"),
        }
        for i in range(2):
            dbg[f"kt{i}"] = nc.dram_tensor(f"dbg_kt{i}", [P, T], BF16, kind="ExternalOutput")
            dbg[f"qpe{i}"] = nc.dram_tensor(f"dbg_qpe{i}", [P, T], BF16, kind="ExternalOutput")
            dbg[f"qpo{i}"] = nc.dram_tensor(f"dbg_qpo{i}", [P, T], BF16, kind="ExternalOutput")
        dbg["rs_all"] = nc.dram_tensor("dbg_rs_all", [32, 1, 512], F32, kind="ExternalOutput")
        dbg["rinv_all"] = nc.dram_tensor("dbg_rinv_all", [32, 1, 512], F32, kind="ExternalOutput")
    with tile.TileContext(nc) as tc:
        _build_kernel_body(
            nc, tc, x_d.ap(), wqkv_d.ap(), wout_d.ap(), out_d.ap(), dbg
        )
    nc.compile()
    _CACHE[key] = nc
    return nc


def _make_in_maps(x, w_qkv, w_out):
    bf = ml_dtypes.bfloat16
    x = np.asarray(x, dtype=np.float32)
    w_qkv = np.asarray(w_qkv, dtype=np.float32)
    w_out = np.asarray(w_out, dtype=np.float32)
    in_maps = []
    for c in range(N_CORES):
        b, g = divmod(c, 2)
        c0 = g * CLOC
        wloc = np.concatenate(
            [
                w_qkv[:, c0:c0 + CLOC],
                w_qkv[:, D + c0:D + c0 + CLOC],
                w_qkv[:, 2 * D + c0:2 * D + c0 + CLOC],
            ],
            axis=1,
        )
        # pre-transpose x on the host: [T, D] -> [d%128, d//128, t]
        xt = np.ascontiguousarray(
            x[b].astype(bf).reshape(T, 8, P).transpose(2, 1, 0)
        )
        in_maps.append({
            "x": xt,
            "w_qkv": np.ascontiguousarray(wloc).astype(bf),
            "w_out": np.ascontiguousarray(w_out[c0:c0 + CLOC]).astype(bf),
        })
    return in_maps


def run(x, w_qkv, w_out, trace=False, debug=False):
    nc = _get_nc(debug)
    in_maps = _make_in_maps(x, w_qkv, w_out)
    res = bass_utils.run_bass_kernel_spmd(
        nc, in_maps, core_ids=list(range(N_CORES)), trace=trace
    )
    y = np.empty((B, T, D), dtype=np.float32)
    for b in range(B):
        y[b] = (res.results[2 * b]["out"].astype(np.float32)
                + res.results[2 * b + 1]["out"].astype(np.float32))
    return y, res


def kernel(x, w_qkv, w_out):
    y, _ = run(x, w_qkv, w_out, trace=False)
    return y


# revision 38
# speedup vs baseline: 2.1562x; 1.0422x over previous
"""Causal self-attention (B=4, T=2048, D=1024, H=16) on 8 TRN2 NeuronCores.

Sharding: core c handles batch b = c//2 and head-group g = c%2 (8 heads each).
Each core computes, for its (b, g):
    qkv_loc = x[b] @ w_qkv[:, cols(g)]          (q|k|v local, 512 cols each)
    att     = causal_attention(q, k, v)          (8 heads, hd=64)
    y_part  = att @ w_out[rows(g), :]            ([2048, 1024] partial)
Host sums the two partial outputs per batch.

All matmuls run in bf16 (host-cast inputs) with fp32 PSUM accumulation.
The attention phase is software-pipelined: AV matmuls lag the score
matmuls by AV_LAG pairs so the in-order PE queue never waits on
exp/mask; QK-projection work for the *next* head-pair is woven between
attention pairs so PE stalls caused by the scalar engine (exp) are
filled with projection matmuls. Softmax normalization is deferred
(rowsums ride along as a ones-column of V); the reciprocal uses the
fast DVE approximation and the partition broadcast runs on the
otherwise-idle GpSimd engine.
"""

from collections import deque

import numpy as np
import ml_dtypes

import concourse.bass as bass
import concourse.mybir as mybir
from concourse import bacc, tile
from concourse import bass_utils
from concourse.masks import make_identity

# Problem constants (hardcoded per contest contract)
B = 4
T = 2048
D = 1024
H = 16
HD = 64
H_LOC = 8               # heads per core
CLOC = H_LOC * HD       # 512 local head dims
P = 128
N_CORES = 8
NTB = T // P            # 16 t-blocks

F32 = mybir.dt.float32
BF16 = mybir.dt.bfloat16

AV_LAG = 4              # pairs the AV matmuls lag behind the score matmuls
NORM_A_LAG = 2          # av-pops before emitting recip+broadcast
NORM_B_LAG = 5          # av-pops before emitting the normalize multiply
WEAVE_EVERY = 5         # emit one projection it-unit per N attention pairs

Exp = mybir.ActivationFunctionType.Exp
MULT = mybir.AluOpType.mult


def _build_kernel_body(nc, tc, x_ap, wqkv_ap, wout_ap, out_ap, dbg=None):
    from contextlib import ExitStack

    ctx = ExitStack()

    # ---------------- constants ----------------
    const = ctx.enter_context(tc.tile_pool(name="const", bufs=1))
    # causal mask helper: wm[p, x] = 1.0 iff p <= x - 384 else 0.0
    wm = const.tile([P, 896], BF16)
    nc.gpsimd.memset(wm, 1.0)
    nc.gpsimd.affine_select(
        out=wm,
        in_=wm,
        compare_op=mybir.AluOpType.is_ge,  # keep where f - p - 384 >= 0
        fill=0.0,
        base=-384,
        channel_multiplier=-1,
        pattern=[[1, 896]],
    )

    big = ctx.enter_context(tc.tile_pool(name="big", bufs=1))
    xT = big.tile([P, 8, T], BF16)                 # [d%128, d//128, t]
    V_aug = big.tile([P, NTB, H_LOC, HD + 1], BF16)  # [j%128, jb, h, dd|1]
    nc.gpsimd.memset(V_aug[:, :, :, HD], 1.0)
    AT = big.tile([P, 4, T], BF16)                 # [(h%2)*64+dd, h//2, t]
    wo_sb = big.tile([P, 4, D], BF16, tag="wo")    # out-proj weights
    # double-buffered per-segment score operands
    kt = [big.tile([P, T], BF16, tag=f"kt{i}", name=f"kt{i}") for i in range(2)]
    qpe = [big.tile([P, T], BF16, tag=f"qpe{i}", name=f"qpe{i}") for i in range(2)]
    qpo = [big.tile([P, T], BF16, tag=f"qpo{i}", name=f"qpo{i}") for i in range(2)]
    for i in range(2):
        nc.gpsimd.memset(qpe[i][64:128, :], 0.0)
        nc.gpsimd.memset(qpo[i][0:64, :], 0.0)

    wv_sb = big.tile([P, 8, CLOC], BF16, tag="wv")

    wqk = wqkv_ap[:, 0:2 * CLOC].rearrange("(o p) c -> p o c", p=P)
    wv = wqkv_ap[:, 2 * CLOC:3 * CLOC].rearrange("(o p) c -> p o c", p=P)

    # ---- interleaved phases A (x load + V proj) + B (QK proj) + C ----
    # Work queues for software pipelining.
    av_q = deque()        # pending AV-pair closures
    norm_q = []           # [countdown, closure] normalize steps
    b_q = deque()         # pending projection it-unit closures

    def pop_norms():
        for e in norm_q:
            e[0] -= 1
        while norm_q and norm_q[0][0] <= 0:
            norm_q.pop(0)[1]()

    def pop_av(n=1):
        for _ in range(n):
            if av_q:
                av_q.popleft()()
                pop_norms()

    emitted = set()

    def pop_b(n=1):
        for _ in range(n):
            if b_q:
                key, fn = b_q.popleft()
                fn()
                emitted.add(key)

    def need(keys):
        while not all(k in emitted for k in keys):
            assert b_q, f"missing producer units: {[k for k in keys if k not in emitted]}"
            pop_b()

    xt_dram = x_ap                     # host pre-transposed: [128, 8, 2048]

    with tc.tile_pool(name="ldw", bufs=4) as ldw, \
         tc.tile_pool(name="att", bufs=1) as att_pool, \
         tc.tile_pool(name="sm", bufs=2) as sm_pool, \
         tc.tile_pool(name="psBC", bufs=1, space="PSUM") as psum:

        norm_idx = [0]
        oa = out_ap.rearrange("(tb p) d -> tb p d", p=P)

        def d_unit(tb, nt):
            py = psum.tile([P, 512], F32, tag="ps_qkv", bufs=2, name="py")
            for k in range(4):
                nc.tensor.matmul(
                    py,
                    AT[:, k, tb * P:(tb + 1) * P],
                    wo_sb[:, k, nt * 512:(nt + 1) * 512],
                    start=(k == 0),
                    stop=(k == 3),
                )
            ysb = att_pool.tile([P, 512], BF16, tag="ysb", bufs=3)
            nc.vector.tensor_copy(ysb, py)
            nc.sync.dma_start(oa[tb, :, nt * 512:(nt + 1) * 512], ysb)

        def v_unit(tb):
            ps = psum.tile([P, CLOC], F32, tag="ps_qkv", bufs=2)
            for k in range(8):
                nc.tensor.matmul(
                    ps,
                    xT[:, k, tb * P:(tb + 1) * P],
                    wv_sb[:, k, :],
                    start=(k == 0),
                    stop=(k == 7),
                )
            nc.vector.tensor_copy(
                V_aug[:, tb, :, 0:HD],
                ps.rearrange("p (h d) -> p h d", h=H_LOC),
            )

        # --- phase B segment: project q (cb=p) and k (cb=4+p) for pair p ---
        def emit_b_segment(p, with_v=False):
            """Queue it-units computing Qp/KT (and V for segment 0)."""
            buf = p % 2
            wcbs = []
            for qk in range(2):            # 0 = q columns, 1 = k columns
                cb = p + 4 * qk
                wcb = ldw.tile([P, 8, P], BF16, tag="wst", name="wcb")
                nc.sync.dma_start(wcb, wqk[:, :, cb * P:(cb + 1) * P])
                wcbs.append(wcb)

            def unit(it, qk):
                wcb = wcbs[qk]
                ps = psum.tile([P, 512], F32, tag="ps_qkv", bufs=2)
                for k in range(8):
                    nc.tensor.matmul(
                        ps,
                        wcb[:, k, :],
                        xT[:, k, it * 512:(it + 1) * 512],
                        start=(k == 0),
                        stop=(k == 7),
                    )
                sl = slice(it * 512, (it + 1) * 512)
                if qk == 0:
                    nc.vector.tensor_copy(qpe[buf][0:64, sl], ps[0:64, :])
                    nc.vector.tensor_copy(qpo[buf][64:128, sl], ps[64:128, :])
                else:
                    nc.vector.tensor_copy(kt[buf][:, sl], ps)

            for it in range(4):
                for qk in range(2):
                    b_q.append((("b", p, qk, it),
                                lambda it=it, qk=qk: unit(it, qk)))
                if with_v:
                    for tb in range(4 * it, 4 * it + 4):
                        b_q.append((("v", tb), lambda tb=tb: v_unit(tb)))

        # --- phase C segment: attention for heads 2p, 2p+1 ---
        def emit_c_segment(p):
            buf = p % 2
            ktb = kt[buf]
            for u2 in range(2):
                h = 2 * p + u2
                hp = qpe[buf] if u2 == 0 else qpo[buf]
                row0 = u2 * 64
                for it in range(4):
                    i0 = it * 512
                    njb = 4 * (it + 1)
                    req = [("b", p, 0, it)] + [("b", p, 1, j) for j in range(it + 1)]
                    if p == 0:
                        req += [("v", tb) for tb in range(njb)]
                    need(req)
                    po = psum.tile([P, 512], F32, tag="ps_o", bufs=2)
                    for jb2 in range(njb // 2):
                        # causal narrowing: diagonal block jb covers only
                        # columns [off, 512) of the i-window (off = jb*128-i0)
                        offs = [max(0, (2 * jb2 + u) * P - i0) for u in range(2)]
                        ps = psum.tile([P, 1024], F32, tag="ps_s", bufs=2)
                        for u in range(2):
                            jb = 2 * jb2 + u
                            off = offs[u]
                            nc.tensor.matmul(
                                ps[:, u * 512 + off:(u + 1) * 512],
                                ktb[:, jb * P:(jb + 1) * P],
                                hp[:, i0 + off:i0 + 512],
                                start=True,
                                stop=True,
                            )
                        es = att_pool.tile([P, 1024], BF16, tag="es", bufs=8)
                        if offs[0] >= 256:
                            # widest-masked pair: exp each block separately
                            for u in range(2):
                                off = offs[u]
                                nc.scalar.activation(
                                    es[:, u * 512 + off:(u + 1) * 512],
                                    ps[:, u * 512 + off:(u + 1) * 512],
                                    Exp, scale=0.125,
                                )
                        else:
                            nc.scalar.activation(es, ps, Exp, scale=0.125)
                        for u in range(2):
                            off = offs[u]
                            if 2 * jb2 + u >= njb - 4:  # diagonal region
                                nc.vector.tensor_tensor(
                                    es[:, u * 512 + off:(u + 1) * 512],
                                    es[:, u * 512 + off:(u + 1) * 512],
                                    wm[:, 384:896 - off],
                                    MULT,
                                )

                        def av(jb2=jb2, es=es, po=po, h=h, njb=njb, offs=offs,
                               row0=row0, p=p, i0=i0, it=it, last=(jb2 == njb // 2 - 1)):
                            for u in range(2):
                                jb = 2 * jb2 + u
                                off = offs[u]
                                nc.tensor.matmul(
                                    po[0:HD + 1, off:512],
                                    V_aug[:, jb, h, :],
                                    es[:, u * 512 + off:(u + 1) * 512],
                                    start=(jb == 0),
                                    stop=(jb == njb - 1),
                                )
                            if last:
                                def norm_a(po=po):
                                    n = norm_idx[0]
                                    norm_idx[0] += 1
                                    rs = sm_pool.tile([1, 512], F32, tag="rs")
                                    rinv = sm_pool.tile([1, 512], F32, tag="rinv")
                                    nc.vector.tensor_copy(rs, po[HD:HD + 1, :])
                                    nc.vector.reciprocal_approx_fast(rinv, rs)
                                    rbb = sm_pool.tile([64, 512], F32, tag="rbb")
                                    nc.gpsimd.partition_broadcast(rbb, rinv)
                                    norm_a.rbb = rbb
                                    if dbg is not None:
                                        nc.sync.dma_start(
                                            dbg["rs_all"].ap()[n], rs
                                        )
                                        nc.sync.dma_start(
                                            dbg["rinv_all"].ap()[n], rinv
                                        )

                                def norm_b(po=po, row0=row0, p=p, i0=i0, it=it):
                                    nc.vector.tensor_tensor(
                                        AT[row0:row0 + 64, p, i0:i0 + 512],
                                        po[0:HD, :],
                                        norm_a.rbb,
                                        MULT,
                                    )
                                    if p == 3 and row0 == 64:
                                        # last head done for this i-range:
                                        # queue the output projection for it
                                        for tb in range(4 * it, 4 * it + 4):
                                            for nt in range(2):
                                                b_q.append((
                                                    ("d", tb, nt),
                                                    lambda tb=tb, nt=nt: d_unit(tb, nt),
                                                ))

                                norm_q.append([NORM_A_LAG, norm_a])
                                norm_q.append([NORM_B_LAG, norm_b])

                        av_q.append(av)
                        while len(av_q) > AV_LAG:
                            pop_av()
                        # weave projection work between attention pairs
                        emit_c_segment.ctr += 1
                        if len(b_q) > 10 or (emit_c_segment.ctr % WEAVE_EVERY == 0):
                            pop_b()

        emit_c_segment.ctr = 0

        # stream in x^T (host pre-transposed) by i-chunk; the first chunk's
        # DMA is issued before everything else so the pipeline starts early
        nc.sync.dma_start(xT[:, :, 0:512], xt_dram[:, :, 0:512])
        emit_b_segment(0, with_v=True)
        nc.sync.dma_start(wv_sb, wv)
        for itc in range(1, 4):
            nc.sync.dma_start(
                xT[:, :, itc * 512:(itc + 1) * 512],
                xt_dram[:, :, itc * 512:(itc + 1) * 512],
            )
        nc.sync.dma_start(wo_sb, wout_ap.rearrange("(o p) n -> p o n", p=P))
        for p in range(4):
            if p < 3:
                emit_b_segment(p + 1)
            emit_c_segment(p)
        # drain: remaining AV pairs, normalizations, and the woven
        # output-projection units they release
        while av_q:
            pop_av()
        while norm_q:
            norm_q.pop(0)[1]()
        pop_b(len(b_q))

    if dbg is not None:
        nc.sync.dma_start(dbg["xT"].ap(), xT)
        nc.sync.dma_start(dbg["V_aug"].ap(), V_aug)
        nc.sync.dma_start(dbg["AT"].ap(), AT)
        for i in range(2):
            nc.sync.dma_start(dbg[f"kt{i}"].ap(), kt[i])
            nc.sync.dma_start(dbg[f"qpe{i}"].ap(), qpe[i])
            nc.sync.dma_start(dbg[f"qpo{i}"].ap(), qpo[i])

    ctx.close()


_CACHE = {}


def _get_nc(debug=False):
    key = ("nc", debug)
    if key in _CACHE:
        return _CACHE[key]
    nc = bacc.Bacc(
        "TRN2",
        target_bir_lowering=False,
        debug=False,
        enable_asserts=False,
        num_devices=N_CORES,
    )
    x_d = nc.dram_tensor("x", [P, 8, T], BF16, kind="ExternalInput")
    wqkv_d = nc.dram_tensor("w_qkv", [D, 3 * CLOC], BF16, kind="ExternalInput")
    wout_d = nc.dram_tensor("w_out", [CLOC, D], BF16, kind="ExternalInput")
    out_d = nc.dram_tensor("out", [T, D], BF16, kind="ExternalOutput")
    dbg = None
    if debug:
        dbg = {
            "xT": nc.dram_tensor("dbg_xT", [P, 8, T], BF16, kind="ExternalOutput"),
            "V_aug": nc.dram_tensor("dbg_V_aug", [P, NTB, H_LOC, HD + 1], BF16, kind="ExternalOutput"),
            "AT": nc.dram_tensor("dbg_AT", [P, 4, T], BF16, kind="ExternalOutput"),
        }
        for i in range(2):
            dbg[f"kt{i}"] = nc.dram_tensor(f"dbg_kt{i}", [P, T], BF16, kind="ExternalOutput")
            dbg[f"qpe{i}"] = nc.dram_tensor(f"dbg_qpe{i}", [P, T], BF16, kind="ExternalOutput")
            dbg[f"qpo{i}"] = nc.dram_tensor(f"dbg_qpo{i}", [P, T], BF16, kind="ExternalOutput")
        dbg["rs_all"] = nc.dram_tensor("dbg_rs_all", [32, 1, 512], F32, kind="ExternalOutput")
        dbg["rinv_all"] = nc.dram_tensor("dbg_rinv_all", [32, 1, 512], F32, kind="ExternalOutput")
    with tile.TileContext(nc) as tc:
        _build_kernel_body(
            nc, tc, x_d.ap(), wqkv_d.ap(), wout_d.ap(), out_d.ap(), dbg
        )
    nc.compile()
    _CACHE[key] = nc
    return nc


def _make_in_maps(x, w_qkv, w_out):
    bf = ml_dtypes.bfloat16
    x = np.asarray(x, dtype=np.float32)
    w_qkv = np.asarray(w_qkv, dtype=np.float32)
    w_out = np.asarray(w_out, dtype=np.float32)
    in_maps = []
    for c in range(N_CORES):
        b, g = divmod(c, 2)
        c0 = g * CLOC
        wloc = np.concatenate(
            [
                w_qkv[:, c0:c0 + CLOC],
                w_qkv[:, D + c0:D + c0 + CLOC],
                w_qkv[:, 2 * D + c0:2 * D + c0 + CLOC],
            ],
            axis=1,
        )
        # pre-transpose x on the host: [T, D] -> [d%128, d//128, t]
        xt = np.ascontiguousarray(
            x[b].astype(bf).reshape(T, 8, P).transpose(2, 1, 0)
        )
        in_maps.append({
            "x": xt,
            "w_qkv": np.ascontiguousarray(wloc).astype(bf),
            "w_out": np.ascontiguousarray(w_out[c0:c0 + CLOC]).astype(bf),
        })
    return in_maps


def run(x, w_qkv, w_out, trace=False, debug=False):
    nc = _get_nc(debug)
    in_maps = _make_in_maps(x, w_qkv, w_out)
    res = bass_utils.run_bass_kernel_spmd(
        nc, in_maps, core_ids=list(range(N_CORES)), trace=trace
    )
    y = np.empty((B, T, D), dtype=np.float32)
    for b in range(B):
        y[b] = (res.results[2 * b]["out"].astype(np.float32)
                + res.results[2 * b + 1]["out"].astype(np.float32))
    return y, res


def kernel(x, w_qkv, w_out):
    y, _ = run(x, w_qkv, w_out, trace=False)
    return y
